# revision 23
# baseline (speedup 1.0000x reference)
"""Trainium2 Bass kernel for a directed-process VGAE (7x GCNConv + inner-product decoder).

Strategy (8 NeuronCores, dst-node sharding, 1024 nodes/core):
  - Host builds the dense normalized adjacency A_hat = D^-1/2 (A+I) D^-1/2 once
    (fp16, [8192, 8192]); core j receives A_hat[jNL:(j+1)NL, :].T  ([8192, 1024]).
  - GCN aggregation A_hat @ (hW) becomes dense matmuls on the PE array with the
    per-core A^T shard SBUF-resident (16 MB fp16); activations flow in transposed
    [channel, node] layout, so biases are per-partition and no transposes are
    ever materialized.
  - s = (A x) Ws + bs, t = (A x) Wt + bt, h1 = relu((A x) W1 + b1) share one
    aggregation of x. Each later layer: project (h @ W, 64 small matmuls),
    aggregate (128 N=512 matmuls), bias(+relu) on the scalar engine.
  - Per-layer AllGather (fp16) replicates h^T across cores; t and h1 share one
    collective. Input loads are spread across the three DMA-capable engines
    (sync/scalar/gpsimd) since a direct DMA blocks its issuing engine.
  - The [8192, 8192] decoder is row-sharded (adj[jNL:(j+1)NL, :] = s @ t_full^T),
    staged through SBUF in fp16 (halves the 256 MB output's write traffic;
    ~7.6e-4 max rel err), with row-blocks emitted between chain layers so the
    PE array has work during the collectives' latency windows.
"""

import sys

sys.path.insert(0, "/opt/trn_rl_repo")

import numpy as np

import concourse.bass as bass
import concourse.bacc as bacc
import concourse.mybir as mybir
import concourse.tile as tile
from concourse.bass_utils import run_bass_kernel_spmd

N = 8192          # nodes
C = 128           # channels
W_CORES = 8
NL = N // W_CORES  # 1024 dst nodes per core
KC = N // 128      # 64 source chunks of 128 nodes

F16 = mybir.dt.float16
F32 = mybir.dt.float32
AF = mybir.ActivationFunctionType

# weight/bias order in the concatenated inputs
W_IDX = {"Ws": 0, "Wt": 1, "W1": 2, "W2": 3, "Wmu": 4, "W5": 5, "W6": 6}


def build_bass():
    nc = bacc.Bacc(num_devices=W_CORES)

    x_in = nc.dram_tensor("x16", [N, C], F16, kind="ExternalInput")
    at_in = nc.dram_tensor("at", [N, NL], F16, kind="ExternalInput")
    w_in = nc.dram_tensor("wcat", [C, 7 * C], F16, kind="ExternalInput")
    b_in = nc.dram_tensor("bcat", [C, 7], F32, kind="ExternalInput")
    adj_out = nc.dram_tensor("adj_out", [NL, N], F16, kind="ExternalOutput")
    h_out = nc.dram_tensor("h_out", [C, NL], F32, kind="ExternalOutput")

    rg = [list(range(W_CORES))]

    with tile.TileContext(nc) as tc:
        with (
            tc.tile_pool(name="big", bufs=1) as big,      # A^T shard
            tc.tile_pool(name="xm", bufs=1) as xm,        # x chunks / hW chunks (shared slot)
            tc.tile_pool(name="hblk", bufs=2) as hblk,    # gathered h^T blocks
            tc.tile_pool(name="gt", bufs=2) as gtp,       # per-layer h^T shard (fp16)
            tc.tile_pool(name="keep", bufs=1) as keep,    # sT, t_full, weights, biases
            tc.tile_pool(name="dec", bufs=5) as dec,      # f32 staging for DMA out
            tc.tile_pool(name="psA", bufs=2, space="PSUM") as psA,
            tc.tile_pool(name="psM", bufs=2, space="PSUM") as psM,
            tc.tile_pool(name="psD", bufs=4, space="PSUM") as psD,
            tc.tile_pool(name="dram", bufs=1, space="DRAM") as dram,
        ):
            # ---- static inputs -> SBUF
            w_sb = keep.tile([C, 7 * C], F16, tag="w")
            nc.sync.dma_start(w_sb[:], w_in[:])
            b_sb = keep.tile([C, 7], F32, tag="b")
            nc.sync.dma_start(b_sb[:], b_in[:])

            x_sb = xm.tile([128, KC * C], F16, tag="xm")
            nc.gpsimd.dma_start(
                x_sb[:].rearrange("p (k c) -> p k c", c=C),
                x_in.rearrange("(k p) c -> p k c", p=128),
            )

            at_sb = big.tile([128, KC * NL], F16, tag="at")
            at_r = at_in.rearrange("(k p) d -> p k d", p=128)
            at_v = at_sb[:].rearrange("p (k d) -> p k d", d=NL)
            for dh in range(2):
                for i, kk in enumerate(range(0, KC, 8)):  # 8 x 1MB per half
                    eng = [nc.sync, nc.scalar, nc.gpsimd][(dh * 8 + i) % 3]
                    eng.dma_start(
                        at_v[:, kk:kk + 8, dh * 512:dh * 512 + 512],
                        at_r[:, kk:kk + 8, dh * 512:dh * 512 + 512],
                    )

            def aggregate(stat_sb):
                """psum halves of (A_shard @ M)^T given stationary chunks M[node, ch].
                Returns (psum halves, last matmul instruction)."""
                halves = []
                last = None
                for dh in range(2):
                    ps = psA.tile([128, 512], F32, tag="agg")
                    for k in range(KC):
                        last = nc.tensor.matmul(
                            ps[:],
                            stat_sb[:, k * C:(k + 1) * C],
                            at_sb[:, k * NL + dh * 512:k * NL + dh * 512 + 512],
                            start=(k == 0),
                            stop=(k == KC - 1),
                        )
                    halves.append(ps)
                return halves, last

            # ---- aggregate x once: g0^T = (A x)^T
            g0_ps, _ = aggregate(x_sb)
            g0 = gtp.tile([128, NL], F16, tag="g")
            for dh, ps in enumerate(g0_ps):
                nc.scalar.activation(g0[:, dh * 512:dh * 512 + 512], ps[:], AF.Identity, bias=0.0)

            # ---- heads from g0 (single-matmul each, contract C)
            def head(widx, relu, out_tile, base=0):
                for dh in range(2):
                    ps = psM.tile([128, 512], F32, tag="m")
                    nc.tensor.matmul(
                        ps[:], w_sb[:, widx * C:(widx + 1) * C],
                        g0[:, dh * 512:dh * 512 + 512], start=True, stop=True,
                    )
                    f = AF.Relu if relu else AF.Identity
                    nc.scalar.activation(
                        out_tile[:, base + dh * 512:base + dh * 512 + 512], ps[:], f,
                        bias=b_sb[:, widx:widx + 1],
                    )

            def allgather(src_tile, tag):
                cin = dram.tile([C, NL], F16, tag=f"cin_{tag}")
                cout = nc.dram_tensor(f"cout_{tag}", [W_CORES * C, NL], F16,
                                      addr_space="Shared")
                nc.sync.dma_start(cin[:], src_tile[:])
                nc.gpsimd.collective_compute(
                    "AllGather", mybir.AluOpType.bypass,
                    replica_groups=rg, ins=[cin.opt()], outs=[cout.ap().opt()],
                )
                return cout

            # t's collective gates the decoder, so it goes first; h1's then
            # hides behind the decoder matmuls. Topology (cc -> t_full loads
            # -> cc) mirrors the originally-validated split-collective order.
            tT = keep.tile([128, NL], F16, tag="t")
            head(W_IDX["Wt"], False, tT)
            cout_t = allgather(tT, "t")

            t_full = keep.tile([128, W_CORES * NL], F16, tag="tf")
            for r in range(W_CORES):
                eng = [nc.sync, nc.scalar, nc.gpsimd][r % 3]
                eng.dma_start(
                    t_full[:, r * NL:(r + 1) * NL], cout_t[r * C:(r + 1) * C, :]
                )

            h1 = gtp.tile([128, NL], F16, tag="g")
            head(W_IDX["W1"], True, h1)
            cout0 = allgather(h1, "h1")

            sT = keep.tile([128, NL], F16, tag="s")
            head(W_IDX["Ws"], False, sT)

            # local decoder columns (own shard, from the local tT) run during
            # the first collective's latency window; the column offset into
            # adj_out is the runtime partition id. The gathered decoder later
            # rewrites these columns with bit-identical values.
            pid_col = nc.gpsimd.partition_id() * NL
            for si in range(8):
                st = dec.tile([128, NL], F16, tag="decst")
                for half in range(2):
                    ps = psD.tile([128, 512], F32, tag="d")
                    nc.tensor.matmul(
                        ps[:], sT[:, si * 128:(si + 1) * 128],
                        tT[:, half * 512:half * 512 + 512], start=True, stop=True,
                    )
                    if half == 0:
                        nc.vector.tensor_copy(st[:, 0:512], ps[:])
                    else:
                        nc.scalar.activation(st[:, 512:1024], ps[:], AF.Copy, bias=0.0)
                nc.gpsimd.dma_start(
                    adj_out[si * 128:(si + 1) * 128, bass.ds(pid_col, NL)],
                    st[:],
                )

            def decoder_block(si, s_src=None):
                """adj rows si*128..+128 = s_chunk @ t_full^T  (16 matmuls N=512)."""
                s_src = sT if s_src is None else s_src
                for quad in range(4):
                    st = dec.tile([128, 2048], F16, tag="decst")
                    for sub in range(4):
                        ti = quad * 4 + sub
                        ps = psD.tile([128, 512], F32, tag="d")
                        nc.tensor.matmul(
                            ps[:], s_src[:, si * 128:(si + 1) * 128],
                            t_full[:, ti * 512:(ti + 1) * 512], start=True, stop=True,
                        )
                        if sub % 2 == 0:
                            nc.vector.tensor_copy(st[:, sub * 512:(sub + 1) * 512], ps[:])
                        else:
                            nc.scalar.activation(
                                st[:, sub * 512:(sub + 1) * 512], ps[:], AF.Copy, bias=0.0
                            )
                    nc.sync.dma_start(
                        adj_out[si * 128:(si + 1) * 128, quad * 2048:(quad + 1) * 2048],
                        st[:],
                    )

            decoder_block(0)
            decoder_block(1)

            # ---- remaining GCN chain: layers use W2, Wmu, W5, W6
            chain = [("W2", True), ("Wmu", False), ("W5", True), ("W6", True)]
            cout_h, h_off = cout0, 0
            for li, (wname, relu) in enumerate(chain):
                widx = W_IDX[wname]
                last_layer = li == len(chain) - 1
                # project: m[node, ch] = h @ W, from gathered h^T blocks
                m_sb = xm.tile([128, KC * C], F16, tag="xm")
                for r in range(W_CORES):
                    hb = hblk.tile([128, NL], F16, tag="hb")
                    eng = [nc.sync, nc.scalar, nc.gpsimd][r % 3]
                    eng.dma_start(
                        hb[:], cout_h[r * C:(r + 1) * C, h_off:h_off + NL]
                    )
                    for grp in range(2):  # 4 chunks per psum bank
                        ps = psM.tile([128, 512], F32, tag="m")
                        for q4 in range(4):
                            q = grp * 4 + q4
                            nc.tensor.matmul(
                                ps[:, q4 * 128:(q4 + 1) * 128],
                                hb[:, q * 128:(q + 1) * 128],
                                w_sb[:, widx * C:(widx + 1) * C],
                                start=True, stop=True,
                            )
                        k0 = r * 8 + grp * 4
                        # alternate the psum->fp16 cast between DVE and ACT
                        if r % 2 == 0:
                            nc.vector.tensor_copy(m_sb[:, k0 * C:(k0 + 4) * C], ps[:])
                        else:
                            nc.scalar.activation(
                                m_sb[:, k0 * C:(k0 + 4) * C], ps[:], AF.Copy, bias=0.0
                            )
                # aggregate + bias(+relu)
                ps_halves, agg_last = aggregate(m_sb)
                if not last_layer:
                    hl = gtp.tile([128, NL], F16, tag="g")
                    for dh, ps in enumerate(ps_halves):
                        nc.scalar.activation(
                            hl[:, dh * 512:dh * 512 + 512], ps[:],
                            AF.Relu if relu else AF.Identity,
                            bias=b_sb[:, widx:widx + 1],
                        )
                    cout_h, h_off = allgather(hl, wname), 0
                else:
                    for dh, ps in enumerate(ps_halves):
                        st = dec.tile([128, 512], F32, tag="decst")
                        nc.scalar.activation(
                            st[:], ps[:], AF.Relu if relu else AF.Identity,
                            bias=b_sb[:, widx:widx + 1],
                        )
                        nc.sync.dma_start(h_out[:, dh * 512:dh * 512 + 512], st[:])
                # decoder blocks fill the PE while this layer's collective runs:
                # give them a data-dependency on this layer's output so the
                # scheduler cannot hoist them into earlier windows
                if not last_layer:
                    ones = keep.tile([128, 1], F32, tag=f"ones{li}")
                    nc.scalar.activation(ones[:], hl[:, 0:1], AF.Identity,
                                         bias=1.0, scale=0.0)
                    s_pin = keep.tile([128, NL], F16, tag=f"spin{li}")
                    nc.vector.tensor_scalar_mul(s_pin[:], sT[:], ones[:])
                    decoder_block(2 + 2 * li, s_src=s_pin)
                    decoder_block(3 + 2 * li, s_src=s_pin)

    nc.compile()
    return nc


_NC = None


def _get_nc():
    global _NC
    if _NC is None:
        _NC = build_bass()
    return _NC


def _host_prep(x, edge_index):
    src = np.asarray(edge_index[0]).astype(np.int64)
    dst = np.asarray(edge_index[1]).astype(np.int64)
    deg = np.bincount(dst, minlength=N).astype(np.float32) + 1.0
    dis = deg ** -0.5
    try:
        from scipy.sparse import coo_matrix
        A = coo_matrix(
            ((dis[dst] * dis[src]).astype(np.float32), (dst, src)), shape=(N, N)
        ).toarray()
    except ImportError:
        A = np.zeros((N, N), np.float32)
        np.add.at(A, (dst, src), (dis[dst] * dis[src]).astype(np.float32))
    idx = np.arange(N)
    A[idx, idx] += dis * dis
    return A.astype(np.float16)


def kernel(**inputs):
    x = np.asarray(inputs["x"], np.float32)
    a16 = _host_prep(x, inputs["edge_index"])
    x16 = np.ascontiguousarray(x.astype(np.float16))
    worder = ["Ws", "Wt", "W1", "W2", "Wmu", "W5", "W6"]
    wcat = np.concatenate(
        [np.asarray(inputs[k], np.float32).astype(np.float16) for k in worder], axis=1
    )
    bcat = np.stack(
        [np.asarray(inputs["b" + k[1:]], np.float32) for k in worder], axis=1
    )

    nc = _get_nc()
    in_maps = []
    for j in range(W_CORES):
        at_j = np.ascontiguousarray(a16[j * NL:(j + 1) * NL, :].T)
        in_maps.append({"x16": x16, "at": at_j, "wcat": wcat, "bcat": bcat})

    res = run_bass_kernel_spmd(nc, in_maps, core_ids=list(range(W_CORES)))
    adj = np.concatenate(
        [res.results[j]["adj_out"].astype(np.float32) for j in range(W_CORES)], axis=0
    )
    h = np.concatenate(
        [res.results[j]["h_out"].T for j in range(W_CORES)], axis=0
    )
    return adj.astype(np.float32), h.astype(np.float32)


# revision 24
# speedup vs baseline: 1.0855x; 1.0855x over previous
"""Trainium2 Bass kernel for a directed-process VGAE (7x GCNConv + inner-product decoder).

Strategy (8 NeuronCores, dst-node sharding, 1024 nodes/core):
  - Host builds the dense normalized adjacency A_hat = D^-1/2 (A+I) D^-1/2 once
    (fp16, [8192, 8192]); core j receives A_hat[jNL:(j+1)NL, :].T  ([8192, 1024]).
  - GCN aggregation A_hat @ (hW) becomes dense matmuls on the PE array with the
    per-core A^T shard SBUF-resident (16 MB fp16); activations flow in transposed
    [channel, node] layout, so biases are per-partition and no transposes are
    ever materialized.
  - s = (A x) Ws + bs, t = (A x) Wt + bt, h1 = relu((A x) W1 + b1) share one
    aggregation of x. Each later layer: project (h @ W, 64 small matmuls),
    aggregate (128 N=512 matmuls), bias(+relu) on the scalar engine.
  - Per-layer AllGather (fp16) replicates h^T across cores; t and h1 share one
    collective. Input loads are spread across the three DMA-capable engines
    (sync/scalar/gpsimd) since a direct DMA blocks its issuing engine.
  - The [8192, 8192] decoder is row-sharded (adj[jNL:(j+1)NL, :] = s @ t_full^T),
    staged through SBUF in fp16 (halves the 256 MB output's write traffic;
    ~7.6e-4 max rel err), with row-blocks emitted between chain layers so the
    PE array has work during the collectives' latency windows.
"""

import sys

sys.path.insert(0, "/opt/trn_rl_repo")

import numpy as np

import concourse.bass as bass
import concourse.bacc as bacc
import concourse.mybir as mybir
import concourse.tile as tile
from concourse.bass_utils import run_bass_kernel_spmd

N = 8192          # nodes
C = 128           # channels
W_CORES = 8
NL = N // W_CORES  # 1024 dst nodes per core
KC = N // 128      # 64 source chunks of 128 nodes

F16 = mybir.dt.float16
F32 = mybir.dt.float32
AF = mybir.ActivationFunctionType

# weight/bias order in the concatenated inputs
W_IDX = {"Ws": 0, "Wt": 1, "W1": 2, "W2": 3, "Wmu": 4, "W5": 5, "W6": 6}


def build_bass():
    nc = bacc.Bacc(num_devices=W_CORES)

    x_in = nc.dram_tensor("x16", [N, C], F16, kind="ExternalInput")
    at_in = nc.dram_tensor("at", [N, NL], F16, kind="ExternalInput")
    w_in = nc.dram_tensor("wcat", [C, 7 * C], F16, kind="ExternalInput")
    b_in = nc.dram_tensor("bcat", [C, 7], F32, kind="ExternalInput")
    adj_out = nc.dram_tensor("adj_out", [NL, N], F16, kind="ExternalOutput")
    h_out = nc.dram_tensor("h_out", [C, NL], F32, kind="ExternalOutput")

    rg = [list(range(W_CORES))]

    with tile.TileContext(nc) as tc:
        with (
            tc.tile_pool(name="big", bufs=1) as big,      # A^T shard
            tc.tile_pool(name="xm", bufs=1) as xm,        # x chunks / hW chunks (shared slot)
            tc.tile_pool(name="hblk", bufs=2) as hblk,    # gathered h^T blocks
            tc.tile_pool(name="gt", bufs=2) as gtp,       # per-layer h^T shard (fp16)
            tc.tile_pool(name="keep", bufs=1) as keep,    # sT, t_full, weights, biases
            tc.tile_pool(name="dec", bufs=5) as dec,      # f32 staging for DMA out
            tc.tile_pool(name="psA", bufs=2, space="PSUM") as psA,
            tc.tile_pool(name="psM", bufs=2, space="PSUM") as psM,
            tc.tile_pool(name="psD", bufs=4, space="PSUM") as psD,
            tc.tile_pool(name="dram", bufs=1, space="DRAM") as dram,
        ):
            # ---- static inputs -> SBUF
            w_sb = keep.tile([C, 7 * C], F16, tag="w")
            nc.sync.dma_start(w_sb[:], w_in[:])
            b_sb = keep.tile([C, 7], F32, tag="b")
            nc.sync.dma_start(b_sb[:], b_in[:])

            x_sb = xm.tile([128, KC * C], F16, tag="xm")
            nc.gpsimd.dma_start(
                x_sb[:].rearrange("p (k c) -> p k c", c=C),
                x_in.rearrange("(k p) c -> p k c", p=128),
            )

            at_sb = big.tile([128, KC * NL], F16, tag="at")
            at_r = at_in.rearrange("(k p) d -> p k d", p=128)
            at_v = at_sb[:].rearrange("p (k d) -> p k d", d=NL)
            for dh in range(2):
                for i, kk in enumerate(range(0, KC, 8)):  # 8 x 1MB per half
                    eng = [nc.sync, nc.scalar, nc.gpsimd][(dh * 8 + i) % 3]
                    eng.dma_start(
                        at_v[:, kk:kk + 8, dh * 512:dh * 512 + 512],
                        at_r[:, kk:kk + 8, dh * 512:dh * 512 + 512],
                    )

            def aggregate(stat_sb):
                """psum halves of (A_shard @ M)^T given stationary chunks M[node, ch].
                Returns (psum halves, last matmul instruction)."""
                halves = []
                last = None
                for dh in range(2):
                    ps = psA.tile([128, 512], F32, tag="agg")
                    for k in range(KC):
                        last = nc.tensor.matmul(
                            ps[:],
                            stat_sb[:, k * C:(k + 1) * C],
                            at_sb[:, k * NL + dh * 512:k * NL + dh * 512 + 512],
                            start=(k == 0),
                            stop=(k == KC - 1),
                        )
                    halves.append(ps)
                return halves, last

            # ---- aggregate x once: g0^T = (A x)^T
            g0_ps, _ = aggregate(x_sb)
            g0 = gtp.tile([128, NL], F16, tag="g")
            for dh, ps in enumerate(g0_ps):
                nc.scalar.activation(g0[:, dh * 512:dh * 512 + 512], ps[:], AF.Identity, bias=0.0)

            # ---- heads from g0 (single-matmul each, contract C)
            def head(widx, relu, out_tile, base=0):
                for dh in range(2):
                    ps = psM.tile([128, 512], F32, tag="m")
                    nc.tensor.matmul(
                        ps[:], w_sb[:, widx * C:(widx + 1) * C],
                        g0[:, dh * 512:dh * 512 + 512], start=True, stop=True,
                    )
                    f = AF.Relu if relu else AF.Identity
                    nc.scalar.activation(
                        out_tile[:, base + dh * 512:base + dh * 512 + 512], ps[:], f,
                        bias=b_sb[:, widx:widx + 1],
                    )

            def allgather(src_tile, tag):
                cin = dram.tile([C, NL], F16, tag=f"cin_{tag}")
                cout = nc.dram_tensor(f"cout_{tag}", [W_CORES * C, NL], F16,
                                      addr_space="Shared")
                nc.sync.dma_start(cin[:], src_tile[:])
                nc.gpsimd.collective_compute(
                    "AllGather", mybir.AluOpType.bypass,
                    replica_groups=rg, ins=[cin.opt()], outs=[cout.ap().opt()],
                )
                return cout

            # t and h1 concatenated in one tile so one collective covers both
            th1 = gtp.tile([128, 2 * NL], F16, tag="th1")
            head(W_IDX["Wt"], False, th1, base=0)
            head(W_IDX["W1"], True, th1, base=NL)

            cin0 = dram.tile([C, 2 * NL], F16, tag="cin0")
            cout0 = nc.dram_tensor("cout0", [W_CORES * C, 2 * NL], F16,
                                   addr_space="Shared")
            nc.sync.dma_start(cin0[:], th1[:])
            nc.gpsimd.collective_compute(
                "AllGather", mybir.AluOpType.bypass,
                replica_groups=rg, ins=[cin0.opt()], outs=[cout0.ap().opt()],
            )

            sT = keep.tile([128, NL], F16, tag="s")
            head(W_IDX["Ws"], False, sT)

            # local decoder columns (own shard, from the local t in th1) run
            # during the first collective's latency window; the column offset
            # into adj_out is the runtime partition id. The gathered decoder
            # later rewrites these columns with bit-identical values.
            pid_col = nc.gpsimd.partition_id() * NL
            for si in range(8):
                st = dec.tile([128, NL], F16, tag="decst")
                for half in range(2):
                    ps = psD.tile([128, 512], F32, tag="d")
                    nc.tensor.matmul(
                        ps[:], sT[:, si * 128:(si + 1) * 128],
                        th1[:, half * 512:half * 512 + 512], start=True, stop=True,
                    )
                    if half == 0:
                        nc.vector.tensor_copy(st[:, 0:512], ps[:])
                    else:
                        nc.scalar.activation(st[:, 512:1024], ps[:], AF.Copy, bias=0.0)
                nc.gpsimd.dma_start(
                    adj_out[si * 128:(si + 1) * 128, bass.ds(pid_col, NL)],
                    st[:],
                )

            t_full = keep.tile([128, W_CORES * NL], F16, tag="tf")
            for r in range(W_CORES):
                eng = [nc.sync, nc.scalar, nc.gpsimd][r % 3]
                eng.dma_start(
                    t_full[:, r * NL:(r + 1) * NL], cout0[r * C:(r + 1) * C, 0:NL]
                )

            def decoder_block(si, s_src=None):
                """adj rows si*128..+128 = s_chunk @ t_full^T  (16 matmuls N=512)."""
                s_src = sT if s_src is None else s_src
                for quad in range(4):
                    st = dec.tile([128, 2048], F16, tag="decst")
                    for sub in range(4):
                        ti = quad * 4 + sub
                        ps = psD.tile([128, 512], F32, tag="d")
                        nc.tensor.matmul(
                            ps[:], s_src[:, si * 128:(si + 1) * 128],
                            t_full[:, ti * 512:(ti + 1) * 512], start=True, stop=True,
                        )
                        if sub % 2 == 0:
                            nc.vector.tensor_copy(st[:, sub * 512:(sub + 1) * 512], ps[:])
                        else:
                            nc.scalar.activation(
                                st[:, sub * 512:(sub + 1) * 512], ps[:], AF.Copy, bias=0.0
                            )
                    nc.sync.dma_start(
                        adj_out[si * 128:(si + 1) * 128, quad * 2048:(quad + 1) * 2048],
                        st[:],
                    )

            decoder_block(0)
            decoder_block(1)

            # ---- remaining GCN chain: layers use W2, Wmu, W5, W6
            chain = [("W2", True), ("Wmu", False), ("W5", True), ("W6", True)]
            cout_h, h_off = cout0, NL  # h1 in cols [NL:2NL]
            for li, (wname, relu) in enumerate(chain):
                widx = W_IDX[wname]
                last_layer = li == len(chain) - 1
                # project: m[node, ch] = h @ W, from gathered h^T blocks
                m_sb = xm.tile([128, KC * C], F16, tag="xm")
                for r in range(W_CORES):
                    hb = hblk.tile([128, NL], F16, tag="hb")
                    eng = [nc.sync, nc.scalar, nc.gpsimd][r % 3]
                    eng.dma_start(
                        hb[:], cout_h[r * C:(r + 1) * C, h_off:h_off + NL]
                    )
                    for grp in range(2):  # 4 chunks per psum bank
                        ps = psM.tile([128, 512], F32, tag="m")
                        for q4 in range(4):
                            q = grp * 4 + q4
                            nc.tensor.matmul(
                                ps[:, q4 * 128:(q4 + 1) * 128],
                                hb[:, q * 128:(q + 1) * 128],
                                w_sb[:, widx * C:(widx + 1) * C],
                                start=True, stop=True,
                            )
                        k0 = r * 8 + grp * 4
                        # alternate the psum->fp16 cast between DVE and ACT
                        if r % 2 == 0:
                            nc.vector.tensor_copy(m_sb[:, k0 * C:(k0 + 4) * C], ps[:])
                        else:
                            nc.scalar.activation(
                                m_sb[:, k0 * C:(k0 + 4) * C], ps[:], AF.Copy, bias=0.0
                            )
                # aggregate + bias(+relu)
                ps_halves, agg_last = aggregate(m_sb)
                if not last_layer:
                    hl = gtp.tile([128, NL], F16, tag="g")
                    for dh, ps in enumerate(ps_halves):
                        nc.scalar.activation(
                            hl[:, dh * 512:dh * 512 + 512], ps[:],
                            AF.Relu if relu else AF.Identity,
                            bias=b_sb[:, widx:widx + 1],
                        )
                    cout_h, h_off = allgather(hl, wname), 0
                else:
                    for dh, ps in enumerate(ps_halves):
                        st = dec.tile([128, 512], F32, tag="decst")
                        nc.scalar.activation(
                            st[:], ps[:], AF.Relu if relu else AF.Identity,
                            bias=b_sb[:, widx:widx + 1],
                        )
                        nc.sync.dma_start(h_out[:, dh * 512:dh * 512 + 512], st[:])
                # decoder blocks fill the PE while this layer's collective runs:
                # give them a data-dependency on this layer's output so the
                # scheduler cannot hoist them into earlier windows
                if not last_layer:
                    ones = keep.tile([128, 1], F32, tag=f"ones{li}")
                    nc.scalar.activation(ones[:], hl[:, 0:1], AF.Identity,
                                         bias=1.0, scale=0.0)
                    s_pin = keep.tile([128, NL], F16, tag=f"spin{li}")
                    nc.vector.tensor_scalar_mul(s_pin[:], sT[:], ones[:])
                    decoder_block(2 + 2 * li, s_src=s_pin)
                    decoder_block(3 + 2 * li, s_src=s_pin)

    nc.compile()
    return nc


_NC = None


def _get_nc():
    global _NC
    if _NC is None:
        _NC = build_bass()
    return _NC


def _host_prep(x, edge_index):
    src = np.asarray(edge_index[0]).astype(np.int64)
    dst = np.asarray(edge_index[1]).astype(np.int64)
    deg = np.bincount(dst, minlength=N).astype(np.float32) + 1.0
    dis = deg ** -0.5
    try:
        from scipy.sparse import coo_matrix
        A = coo_matrix(
            ((dis[dst] * dis[src]).astype(np.float32), (dst, src)), shape=(N, N)
        ).toarray()
    except ImportError:
        A = np.zeros((N, N), np.float32)
        np.add.at(A, (dst, src), (dis[dst] * dis[src]).astype(np.float32))
    idx = np.arange(N)
    A[idx, idx] += dis * dis
    return A.astype(np.float16)


def kernel(**inputs):
    x = np.asarray(inputs["x"], np.float32)
    a16 = _host_prep(x, inputs["edge_index"])
    x16 = np.ascontiguousarray(x.astype(np.float16))
    worder = ["Ws", "Wt", "W1", "W2", "Wmu", "W5", "W6"]
    wcat = np.concatenate(
        [np.asarray(inputs[k], np.float32).astype(np.float16) for k in worder], axis=1
    )
    bcat = np.stack(
        [np.asarray(inputs["b" + k[1:]], np.float32) for k in worder], axis=1
    )

    nc = _get_nc()
    in_maps = []
    for j in range(W_CORES):
        at_j = np.ascontiguousarray(a16[j * NL:(j + 1) * NL, :].T)
        in_maps.append({"x16": x16, "at": at_j, "wcat": wcat, "bcat": bcat})

    res = run_bass_kernel_spmd(nc, in_maps, core_ids=list(range(W_CORES)))
    adj = np.concatenate(
        [res.results[j]["adj_out"].astype(np.float32) for j in range(W_CORES)], axis=0
    )
    h = np.concatenate(
        [res.results[j]["h_out"].T for j in range(W_CORES)], axis=0
    )
    return adj.astype(np.float32), h.astype(np.float32)
